# revision 72
# baseline (speedup 1.0000x reference)
"""CrossCovarianceAttn Trainium2 kernel.

Data-parallel over B=8 across 8 NeuronCores; each core runs the full model on
one batch element.

Core restructure vs the straightforward pipeline: attn@v and the output
projection are folded all the way back into x.  With
B[(h,e),co] = sum_d attn_h[d,e] * wp[co, 96h+d]  (the per-head attn/proj fold)
and v = Wv x, the output is  y = x @ M,  M = Wv^T @ B  (768x768 per batch).
This deletes the entire v-projection GEMM (768x768x4096) and its psum drains;
the big bf16 GEMM left is y = x @ M (plus the fp8 qk projection).  Wv is
consumed in its raw HBM layout (vfeat on partitions) - no transpose.  B is
built directly in dense 128-row layout by offsetting matmul outputs to the
strip partition ranges, so no DMA relayout is needed.

Numerics: q/k projection and the per-head covariance/Gram matmuls run in
fp8e4m3 with perf_mode=DoubleRow (0.5 cyc/row) - safe because q,k are
l2-normalized over the token dim downstream, which cancels the fp8 scaling,
and softmax logits are small (|logit| <= temperature by Cauchy-Schwarz, so no
max-subtraction).  w_q/w_k are scaled x64 into fp8 range; the qk psum->fp8
cast divides it back via C_QK.  Everything on the v/proj path stays bf16.

Engine layout: PE does transposes + qk (fp8 DR) + covariance (fp8 DR,
accumulated over 4-tile rounds) + B/M builds + the y GEMM.  The 12 qk psum
chains per tile are single-bank through a 6-deep ring so the PE never stalls
on a drain; drains are split DVE/ACT by comparative advantage (DVE gets the
2x-mode bf16 xT drains + cov adds, ACT the fp8 ones), and the steady-state
fp8 xT casts run on the otherwise idle GpSimd engine.  The y bias-add runs
as tensor_tensor-add on DVE (ACT has no tensor_tensor).  x/wproj/wv loads
are GpSimd SWDGE cast-DMAs (fp32->bf16 in flight) with x issued decoupled
from compute so the queue never blocks; w_qk loads are fp32r on the SP/ACT
HWDGE queues (parallel with the x stream) and transposed at 1.5 cyc/row; the
identity comes from a NEFF-embedded constant; stores are issued on SP with
the final piece split across both HWDGE queues.  The rk norm reciprocals
reach the softmax free dim via per-head PE column-transposes to a partition-0
row plus GpSimd partition_broadcast (no DRAM round-trip), and the softmax
tail runs per-head (exp with accum_out) so the B matmuls start while later
heads are still in flight.  ACT's activation-table warmup ends on the Sqrt
set and a dummy Exp prefetches the Exp set off the critical path.
"""
import os
import sys

sys.path.insert(0, "/opt/trn_rl_repo")

import numpy as np

import concourse.bass as bass
import concourse.mybir as mybir
import concourse.tile as tile
from concourse import bacc
from concourse.bass_utils import run_bass_kernel_spmd
from concourse.masks import make_identity

FP32 = mybir.dt.float32
FP32R = mybir.dt.float32r
BF16 = mybir.dt.bfloat16
FP8 = mybir.dt.float8e4
DR = mybir.MatmulPerfMode.DoubleRow

N_TOK = 4096
C = 768
H = 8
HD = 96
C3 = 3 * C
TOK_TILE = 512
N_TILES = N_TOK // TOK_TILE
CHUNKS = TOK_TILE // 128
KK = C // 128
EPS = 1e-12

S_W = 64.0           # w_q/w_k -> fp8 scale
C_QK = 26.0 / 35.5   # qk psum (scaled x64) -> fp8 scale

_CACHED_NC = None


def _vt_strips():
    """Strips (m, p0, run, h, d0): dense partition p of block m holds
    v-feature c = 128m + p = 96h + d -> per-head partition d, head h."""
    strips = []
    for m in range(KK):
        c0 = 128 * m
        p = 0
        while p < 128:
            h, d = divmod(c0 + p, HD)
            run = min(128 - p, HD - d)
            strips.append((m, p, run, h, d))
            p += run
    return strips


def build_nc():
    nc = bacc.Bacc("TRN2", target_bir_lowering=False, debug=False, num_devices=8)

    ident_d = nc.inline_tensor(np.eye(128, dtype=np.float32),
                               name="ident_const").ap()
    x_d = nc.dram_tensor("x", (N_TOK, C), FP32R, kind="ExternalInput").ap()
    wqkv_d = nc.dram_tensor("w_qkv", (C3, C), FP32R, kind="ExternalInput").ap()
    temp_d = nc.dram_tensor("temperature", (H, 1, 1), FP32, kind="ExternalInput").ap()
    wproj_d = nc.dram_tensor("w_proj", (C, C), FP32R, kind="ExternalInput").ap()
    bproj_d = nc.dram_tensor("b_proj", (C,), FP32, kind="ExternalInput").ap()
    out_d = nc.dram_tensor("out", (N_TOK, C), FP32, kind="ExternalOutput").ap()

    with tile.TileContext(nc) as tc:
        _build(tc, nc, x_d, wqkv_d, temp_d, wproj_d, bproj_d, out_d, ident_d)
    nc.compile()
    return nc


def _build(tc, nc, x_d, wqkv_d, temp_d, wproj_d, bproj_d, out_d, ident_d):
    import contextlib

    ctx = contextlib.ExitStack()
    with ctx:
        singles = ctx.enter_context(tc.tile_pool(name="singles", bufs=1))
        dram = ctx.enter_context(tc.tile_pool(name="dram", bufs=1, space="DRAM"))

        # identity loaded from a NEFF-embedded constant on the idle SP
        # HWDGE queue: no Pool time, ready by the time x(0) lands
        ident_f32 = singles.tile([128, 128], FP32)
        nc.sync.dma_start(ident_f32, ident_d)
        ident = singles.tile([128, 128], BF16)
        nc.vector.tensor_copy(ident, ident_f32)
        ident_r = singles.tile([128, 128], FP32R)
        nc.vector.tensor_copy(ident_r, ident_f32)
        ident96f = ident_f32[0:HD, 0:HD]

        # act-table warmup: end on the Sqrt set (phase-2 order is sqrt->exp;
        # a dummy exp there prefetches the Exp set off the critical path)
        warm = singles.tile([1, 1], FP32)
        nc.vector.memset(warm, 0.5)
        nc.scalar.activation(warm, warm, mybir.ActivationFunctionType.Exp)
        nc.scalar.sqrt(warm, warm)

        cg_accum = singles.tile([HD, H, 288], FP32)
        nc.vector.memset(cg_accum, 0.0)
        attn_bf = singles.tile([HD, H, HD], BF16)
        # xT (feature-major bf16 x) for the whole batch element: consumed by
        # the fused projection GEMM at the end, so it persists all of phase 1.
        xT_bf = singles.tile([128, KK, N_TOK], BF16)
        wv_bf = singles.tile([128, KK, C], BF16)      # w_v raw (vfeat-part)
        w_projT = singles.tile([HD, H, C], BF16)      # per-head wp^T

        # ---------------- phase 0 + 1 ----------------
        # w_qk plain layout: w_qk_f8[p][:, kk, j] = 64 * w_qkv[512p+j, 128kk+:]
        qkp_holder = []
        with tc.tile_pool(name="wqk_pool", bufs=1) as wqk_pool, \
             tc.tile_pool(name="xin", bufs=3) as xin, \
             tc.tile_pool(name="xf8", bufs=3) as xf8, \
             tc.tile_pool(name="ps_tr", bufs=2, space="PSUM") as ps_tr, \
             tc.tile_pool(name="ps_one", bufs=6, space="PSUM") as ps_one:
            w_qk_f8 = [wqk_pool.tile([128, KK, 512], FP8, name=f"wqk{p}")
                       for p in range(3)]

            # w_qk loads: fp32r on the two HWDGE queues, parallel with the
            # SWDGE x stream; staging pool scoped to the fill so its SBUF is
            # recycled into qk_t buffers for all 8 tiles
            wload_cm = tc.tile_pool(name="wload", bufs=2)
            wload = wload_cm.__enter__()
            w_blks = {}
            for g in range(6):
                w_blk = wload.tile([128, 2, C], FP32R, name="w_blk",
                                   tag=f"w{g % 2}", bufs=3)
                eng = nc.sync if g % 2 == 0 else nc.scalar
                eng.dma_start(
                    w_blk,
                    wqkv_d[g * 256:(g + 1) * 256, :].rearrange(
                        "(b p) f -> p b f", p=128))
                w_blks[g] = w_blk

            b_all = singles.tile([128, C], FP32)
            nc.sync.dma_start(
                b_all, bass.AP(tensor=bproj_d.tensor, offset=bproj_d.offset,
                               ap=[[0, 128], [1, C]]))
            temp_all = singles.tile([HD, H], FP32)
            nc.sync.dma_start(
                temp_all, bass.AP(tensor=temp_d.tensor, offset=temp_d.offset,
                                  ap=[[0, HD], [1, H]]))

            x_ts = {}

            def issue_x(t):
                """x cast-DMA issue only, so the SWDGE queue never blocks
                behind Pool compute ops."""
                t0 = t * TOK_TILE
                x_t = xin.tile([128, CHUNKS, C], BF16, name="x_t")
                if t < 2:
                    # split by kk-pair so each transpose group starts as its
                    # columns arrive
                    for kp in range(3):
                        nc.gpsimd.dma_start(
                            x_t[:, :, kp * 256:(kp + 1) * 256],
                            x_d[t0:t0 + TOK_TILE,
                                kp * 256:(kp + 1) * 256].rearrange(
                                "(c p) f -> p c f", p=128))
                else:
                    nc.gpsimd.dma_start(
                        x_t, x_d[t0:t0 + TOK_TILE, :].rearrange(
                            "(c p) f -> p c f", p=128))
                x_ts[t] = x_t

            def xstage(t):
                """PE transpose + bf16/fp8 drains for an already-loading x."""
                t0 = t * TOK_TILE
                x_t = x_ts.pop(t)
                xT_f8 = xf8.tile([128, KK, TOK_TILE], FP8, name="xT_f8")
                for kp in range(KK // 2):
                    xps = ps_tr.tile([128, 1024], BF16, name="xps", tag="tr")
                    for j in range(2):
                        kk = 2 * kp + j
                        for c in range(CHUNKS):
                            nc.tensor.transpose(
                                xps[:, j * 512 + c * 128:j * 512 + (c + 1) * 128],
                                x_t[:, c, kk * 128:(kk + 1) * 128], ident)
                    xv = xps.rearrange("p (k f) -> p k f", k=2)
                    nc.vector.tensor_copy(
                        xT_bf[:, 2 * kp:2 * kp + 2, t0:t0 + TOK_TILE], xv)
                    if t < 3:
                        # fill: kp1 on DVE so the first qk chains aren't
                        # ACT-serialized
                        if kp == 1:
                            nc.vector.tensor_copy(
                                xT_f8[:, 2:4, :], xv)
                        else:
                            nc.scalar.copy(
                                xT_f8[:, 2 * kp:2 * kp + 2, :], xv)
                    else:
                        nc.gpsimd.tensor_copy(
                            xT_f8[:, 2 * kp:2 * kp + 2, :],
                            xT_bf[:, 2 * kp:2 * kp + 2, t0:t0 + TOK_TILE])
                return xT_f8

            state = {"qk_pair": []}
            sq = singles.tile([HD, H, 2], FP32)
            scrd = singles.tile([HD, HD], FP32)

            def mmstage(t, xT_f8, mid_hook=None):
                """qk matmuls + drains, covariance every fourth tile.
                12 single-bank psum chains per tile through a 6-deep ring so
                the PE never stalls on a drain."""
                qk_t = qkp_holder[0].tile([128, CHUNKS, 1536], FP8, name="qk_t")

                # comparative advantage: ACT is cheaper on fp8-out psum
                # drains, DVE on bf16 (2x mode) + the cov adds. During fill
                # (t<3) ACT also does the fp8 casts, so DVE takes more.
                if t < 3:
                    dve_drains = {(c, p) for c in range(CHUNKS)
                                  for p in range(3) if (c * 3 + p) % 12 in
                                  (0, 2, 4, 6, 8, 10, 11)}
                else:
                    dve_drains = {(0, 0), (2, 0), (1, 1), (3, 2)}
                for p in range(3):
                    if p == 2 and mid_hook is not None:
                        mid_hook()
                    for c in range(CHUNKS):
                        ps1 = ps_one.tile([128, 512], FP32, name="ps1",
                                          tag="one")
                        for i in range(KK // 2):
                            nc.tensor.matmul(
                                ps1,
                                xT_f8[:, 2 * i:2 * i + 2,
                                      c * 128:(c + 1) * 128],
                                w_qk_f8[p][:, 2 * i:2 * i + 2, :],
                                start=(i == 0), stop=(i == KK // 2 - 1),
                                perf_mode=DR)
                        dst = qk_t[:, c, p * 512:(p + 1) * 512]
                        if (c, p) in dve_drains:
                            nc.vector.tensor_scalar_mul(dst, ps1, C_QK)
                        else:
                            nc.scalar.mul(dst, ps1, C_QK)

                # covariance + Gram over four tiles at a time: per head
                # [Gq | Gk | C] = [q'q | k'k | q'k], DoubleRow chunk pairs
                state["qk_pair"].append(qk_t)
                if t == N_TILES - 1:
                    qk_pair = state["qk_pair"]
                    for h in range(H):
                        cg_ps = ps_one.tile([HD, 288], FP32, name="cg_ps",
                                            tag="one")
                        np_ = 2 * len(qk_pair)
                        for i in range(np_):
                            qkx = qk_pair[i // 2]
                            lo = (i % 2) * 2
                            q_sl = qkx[:, lo:lo + 2, HD * h:HD * h + HD]
                            k_sl = qkx[:, lo:lo + 2,
                                       C + HD * h:C + HD * h + HD]
                            nc.tensor.matmul(
                                cg_ps[:, 0:HD], q_sl, q_sl,
                                start=(i == 0), stop=False, perf_mode=DR)
                            nc.tensor.matmul(
                                cg_ps[:, HD:2 * HD], k_sl, k_sl,
                                start=False, stop=False, perf_mode=DR)
                            nc.tensor.matmul(
                                cg_ps[:, 2 * HD:3 * HD], q_sl, k_sl,
                                start=False, stop=(i == np_ - 1),
                                perf_mode=DR)
                        nc.vector.tensor_add(
                            cg_accum[:, h, :], cg_ps, cg_accum[:, h, :])
                    state["qk_pair"] = []

            def wprep(grp):
                """w_q/w_k rows 256*grp..256*grp+256 -> w_qk_f8 (x64).
                2 kk-blocks (4 transposes) per psum, one merged drain each."""
                w_blk = w_blks.pop(grp)
                goff = (grp % 2) * 256
                for kp in range(KK // 2):
                    tps = ps_one.tile([128, 512], FP32R, name="wps",
                                      tag="one")
                    for j in range(2):
                        kk = 2 * kp + j
                        for b in range(2):
                            nc.tensor.transpose(
                                tps[:, j * 256 + b * 128:
                                    j * 256 + (b + 1) * 128],
                                w_blk[:, b, kk * 128:(kk + 1) * 128], ident_r)
                    dst = w_qk_f8[grp // 2][
                        :, 2 * kp:2 * kp + 2, goff:goff + 256]
                    src = tps.rearrange("p (k f) -> p k f", k=2)
                    if (grp + kp) % 2 == 0:
                        nc.vector.tensor_scalar_mul(dst, src, S_W)
                    else:
                        nc.scalar.mul(dst, src, S_W)

            wp_all = singles.tile([128, KK, C], BF16)

            def wproj_tr(n):
                """w_proj rows 128n..128n+128 -> w_projT[:, :, n*128:...]."""
                wp_blk = wp_all[:, n, :]
                for hg in range(2):
                    ps_w = ps_tr.tile([128, 512], BF16, name="psw", tag="tr")
                    for hh in range(4):
                        h = 4 * hg + hh
                        nc.tensor.transpose(
                            ps_w[0:HD, hh * 128:(hh + 1) * 128],
                            wp_blk[:, h * HD:(h + 1) * HD], ident)
                    src = ps_w[0:HD, :].rearrange("p (h f) -> p h f", h=4)
                    dst = w_projT[:, 4 * hg:4 * hg + 4, n * 128:(n + 1) * 128]
                    if hg == 0:
                        nc.vector.tensor_copy(dst, src)
                    else:
                        nc.scalar.copy(dst, src)

            # x0..x2 issued up front (before any other Pool work) so the
            # first transfer starts as early as possible
            issue_x(0)
            issue_x(1)
            issue_x(2)
            xts = {}
            wprep(0)
            wprep(1)
            xts[0] = xstage(0)
            xts[1] = xstage(1)
            wprep(2)
            wprep(3)
            wprep(4)
            wprep(5)
            wload_cm.__exit__(None, None, None)

            # w staging freed: its SBUF becomes qk_t buffers for all 8
            # tiles, so the covariance runs as one single-round accumulate
            qkp_cm = tc.tile_pool(name="qkp", bufs=8)
            qkp_holder.append(qkp_cm.__enter__())
            for t in range(N_TILES):
                mmstage(t, xts.pop(t))
                if t + 3 < N_TILES:
                    issue_x(t + 3)
                if t == 4:
                    # w_proj load: consumed by the transposes after the loop
                    nc.gpsimd.dma_start(
                        wp_all, wproj_d.rearrange("(b p) f -> p b f", p=128))
                if t == 5:
                    # w_v load: only needed for the M build at the very end
                    nc.gpsimd.dma_start(
                        wv_bf, wqkv_d[2 * C:3 * C, :].rearrange(
                            "(b p) f -> p b f", p=128))
                if t + 2 < N_TILES:
                    xts[t + 2] = xstage(t + 2)

            # w_proj transposes fill the PE gap while the softmax chain runs
            for n in range(KK):
                wproj_tr(n)

            # ---------------- phase 2: norms + per-head softmax ----------
            identb = ident96f[:, None, None, :].to_broadcast((HD, H, 2, HD))
            scr2 = singles.tile([HD, H, 2, HD], FP32)
            nc.vector.tensor_tensor(
                scr2, cg_accum[:, :, 0:2 * HD].rearrange(
                    "d h (two e) -> d h two e", two=2),
                identb, mybir.AluOpType.mult)
            nc.vector.reduce_sum(sq[:, :, :, None], scr2,
                                 axis=mybir.AxisListType.X)
            nrm = singles.tile([HD, H, 2], FP32)
            nc.scalar.sqrt(nrm, sq)
            # prefetch the Exp act table while DVE works on the norms
            nc.scalar.activation(warm, warm, mybir.ActivationFunctionType.Exp)
            nc.vector.tensor_scalar_max(nrm, nrm, EPS)
            rnorm = singles.tile([HD, H, 2], FP32)
            nc.vector.reciprocal(rnorm, nrm)
            rq = singles.tile([HD, H], FP32)
            nc.vector.tensor_tensor(rq, rnorm[:, :, 0], temp_all,
                                    mybir.AluOpType.mult)

            # rk to the free dim: tiny PE transpose + per-head GpSimd
            # partition broadcast (no DRAM round trip)
            # each head's rnorm column -> a partition-0 row segment (the
            # broadcast ISA op only reads partition 0)
            rk_ps = [ps_tr.tile([128, 512], FP32, name=f"rk_ps{i}", tag="tr")
                     for i in range(2)]
            for h in range(H):
                nc.tensor.transpose(
                    rk_ps[h // 4][0:1, (h % 4) * HD:(h % 4 + 1) * HD],
                    rnorm[:, h:h + 1, 1], ident96f)
            rk_row = singles.tile([1, H * HD], FP32)
            nc.vector.tensor_copy(rk_row[:, 0:4 * HD], rk_ps[0][0:1, 0:4 * HD])
            nc.vector.tensor_copy(rk_row[:, 4 * HD:], rk_ps[1][0:1, 0:4 * HD])
            rk_all = singles.tile([HD, H, HD], FP32)

            # fully per-head tail: broadcast -> logits -> exp -> normalize,
            # so head 0's B matmuls start while later heads are in flight
            attL = singles.tile([HD, H, HD], FP32)
            sea = singles.tile([HD, H, 1], FP32)
            rsea = singles.tile([HD, H, 1], FP32)
            for h in range(H):
                if h % 4 == 0:
                    nc.gpsimd.partition_broadcast(
                        rk_all[:, h:h + 4, :],
                        rk_row[:, h * HD:(h + 4) * HD])
                nc.vector.tensor_tensor(
                    attL[:, h, :], cg_accum[:, h, 2 * HD:3 * HD],
                    rq[:, h, None].to_broadcast((HD, HD)),
                    mybir.AluOpType.mult)
                nc.vector.tensor_tensor(
                    attL[:, h, :], attL[:, h, :], rk_all[:, h, :],
                    mybir.AluOpType.mult)
                nc.scalar.activation(
                    attL[:, h, :], attL[:, h, :],
                    mybir.ActivationFunctionType.Exp,
                    accum_out=sea[:, h, :])
                nc.vector.reciprocal(rsea[:, h, :], sea[:, h, :])
                nc.vector.tensor_tensor(
                    attn_bf[:, h, :], attL[:, h, :],
                    rsea[:, h, :].to_broadcast((HD, HD)),
                    mybir.AluOpType.mult)
            qkp_cm.__exit__(None, None, None)

        # ---------------- phase 3: B, M = wv^T B, y = x @ M ---------------
        strips = _vt_strips()
        with tc.tile_pool(name="wpp", bufs=1) as wpp, \
             tc.tile_pool(name="yp", bufs=2) as yp, \
             tc.tile_pool(name="ps_b", bufs=2, space="PSUM") as ps_b, \
             tc.tile_pool(name="ps_m", bufs=2, space="PSUM") as ps_m, \
             tc.tile_pool(name="ps_y", bufs=4, space="PSUM") as ps_y:
            # B built directly in dense 128-row layout: per (m, half) one psum
            # whose partition ranges are filled by per-head strip matmuls
            b128 = wpp.tile([128, KK, C], BF16)
            m_sb = wpp.tile([128, KK, C], BF16)
            last_h_of_m = {}
            for (m, p0, run, h, d0) in strips:
                last_h_of_m[m] = h
            def legal_pieces(p0, run, d0):
                """PE col tile positions are restricted: <=32-row tiles can
                sit at 0/32/64/96, <=64 at 0/64, bigger only at 0."""
                if run > 64 and p0 not in (0,):
                    assert p0 == 32 and run == 96
                    return [(32, 32, d0), (64, 64, d0 + 32)]
                if 32 < run <= 64 and p0 not in (0, 64):
                    return [(p0, 32, d0), (p0 + 32, run - 32, d0 + 32)]
                return [(p0, run, d0)]

            for oi, (off, width) in enumerate(((0, 512), (512, 256))):
                open_ps = {}
                for si, (m, p0, run, h, d0) in enumerate(strips):
                    if m not in open_ps:
                        open_ps[m] = ps_b.tile([128, 512], FP32, name="bps")
                    for (pp, rr, dd) in legal_pieces(p0, run, d0):
                        nc.tensor.matmul(
                            open_ps[m][pp:pp + rr, :width],
                            attn_bf[:, h, dd:dd + rr],
                            w_projT[:, h, off:off + width],
                            start=True, stop=True,
                            tile_position=(0, pp))
                    if h == last_h_of_m[m]:
                        bps = open_ps.pop(m)
                        if (m + oi) % 2 == 0:
                            nc.vector.tensor_copy(
                                b128[:, m, off:off + width], bps[:, :width])
                        else:
                            nc.scalar.copy(
                                b128[:, m, off:off + width], bps[:, :width])

                # M half: M = wv^T @ b128 over 6 vfeat blocks
                for n in range(KK):
                    mps = ps_m.tile([128, 512], FP32, name="mps")
                    for b in range(KK):
                        nc.tensor.matmul(
                            mps[:, :width],
                            wv_bf[:, b, n * 128:(n + 1) * 128],
                            b128[:, b, off:off + width],
                            start=(b == 0), stop=(b == KK - 1))
                    if (n + oi) % 2 == 0:
                        nc.vector.tensor_copy(
                            m_sb[:, n, off:off + width], mps[:, :width])
                    else:
                        nc.scalar.copy(
                            m_sb[:, n, off:off + width], mps[:, :width])

            # ---------------- y = x @ M + b ------------------------------
            for t in range(N_TILES):
                for piece in range(2):
                    last = (t == N_TILES - 1 and piece == 1)
                    t0 = t * TOK_TILE + piece * 256
                    y_t = yp.tile([128, 2, C], FP32, name="y_t")
                    for c in range(2):
                        cc = t * CHUNKS + piece * 2 + c
                        for oi, (off, width) in enumerate(
                                ((0, 512), (512, 256))):
                            yps = ps_y.tile([128, 512], FP32, name="yps")
                            for kk in range(KK):
                                nc.tensor.matmul(
                                    yps[:, :width],
                                    xT_bf[:, kk, cc * 128:(cc + 1) * 128],
                                    m_sb[:, kk, off:off + width],
                                    start=(kk == 0), stop=(kk == KK - 1))
                            nc.vector.tensor_tensor(
                                y_t[:, c, off:off + width],
                                yps[:, :width],
                                b_all[:, off:off + width],
                                mybir.AluOpType.add)
                    if last:
                        # split the final store across both HWDGE queues,
                        # one 128-token row-block each, issued per sub-block
                        # so the tail is one drain + one small DMA
                        nc.sync.dma_start(
                            out_d[t0:t0 + 128, :].rearrange(
                                "(c p) f -> p c f", p=128),
                            y_t[:, 0:1, :])
                        nc.scalar.dma_start(
                            out_d[t0 + 128:t0 + 256, :].rearrange(
                                "(c p) f -> p c f", p=128),
                            y_t[:, 1:2, :])
                    else:
                        nc.sync.dma_start(
                            out_d[t0:t0 + 256, :].rearrange(
                                "(c p) f -> p c f", p=128),
                            y_t)


def _get_nc():
    global _CACHED_NC
    if _CACHED_NC is None:
        _CACHED_NC = build_nc()
    return _CACHED_NC


def kernel(x, w_qkv, temperature, w_proj, b_proj):
    nc = _get_nc()
    x = np.ascontiguousarray(np.asarray(x, dtype=np.float32))
    in_maps = []
    for b in range(8):
        in_maps.append({
            "x": x[b],
            "w_qkv": np.asarray(w_qkv, dtype=np.float32),
            "temperature": np.asarray(temperature, dtype=np.float32),
            "w_proj": np.asarray(w_proj, dtype=np.float32),
            "b_proj": np.asarray(b_proj, dtype=np.float32),
        })
    res = run_bass_kernel_spmd(nc, in_maps, core_ids=list(range(8)))
    return np.stack([r["out"] for r in res.results], axis=0)


# revision 76
# speedup vs baseline: 1.0025x; 1.0025x over previous
"""CrossCovarianceAttn Trainium2 kernel.

Data-parallel over B=8 across 8 NeuronCores; each core runs the full model on
one batch element.

Core restructure vs the straightforward pipeline: attn@v and the output
projection are folded all the way back into x.  With
B[(h,e),co] = sum_d attn_h[d,e] * wp[co, 96h+d]  (the per-head attn/proj fold)
and v = Wv x, the output is  y = x @ M,  M = Wv^T @ B  (768x768 per batch).
This deletes the entire v-projection GEMM (768x768x4096) and its psum drains;
the big bf16 GEMM left is y = x @ M (plus the fp8 qk projection).  Wv is
consumed in its raw HBM layout (vfeat on partitions) - no transpose.  B is
built directly in dense 128-row layout by offsetting matmul outputs to the
strip partition ranges, so no DMA relayout is needed.

Numerics: q/k projection and the per-head covariance/Gram matmuls run in
fp8e4m3 with perf_mode=DoubleRow (0.5 cyc/row) - safe because q,k are
l2-normalized over the token dim downstream, which cancels the fp8 scaling,
and softmax logits are small (|logit| <= temperature by Cauchy-Schwarz, so no
max-subtraction).  w_q/w_k are scaled x64 into fp8 range; the qk psum->fp8
cast divides it back via C_QK.  Everything on the v/proj path stays bf16.

Engine layout: PE does transposes + qk (fp8 DR) + covariance (fp8 DR,
accumulated over 4-tile rounds) + B/M builds + the y GEMM.  The 12 qk psum
chains per tile are single-bank through a 6-deep ring so the PE never stalls
on a drain; drains are split DVE/ACT by comparative advantage (DVE gets the
2x-mode bf16 xT drains + cov adds, ACT the fp8 ones), and the steady-state
fp8 xT casts run on the otherwise idle GpSimd engine.  The y bias-add runs
as tensor_tensor-add on DVE (ACT has no tensor_tensor).  x/wproj/wv loads
are GpSimd SWDGE cast-DMAs (fp32->bf16 in flight) with x issued decoupled
from compute so the queue never blocks; w_qk loads are fp32r on the SP/ACT
HWDGE queues (parallel with the x stream) and transposed at 1.5 cyc/row; the
identity comes from a NEFF-embedded constant; stores are issued on SP with
the final piece split across both HWDGE queues.  The rk norm reciprocals
reach the softmax free dim via per-head PE column-transposes to a partition-0
row plus GpSimd partition_broadcast (no DRAM round-trip), and the softmax
tail runs per-head (exp with accum_out) so the B matmuls start while later
heads are still in flight.  ACT's activation-table warmup ends on the Sqrt
set and a dummy Exp prefetches the Exp set off the critical path.
"""
import os
import sys

sys.path.insert(0, "/opt/trn_rl_repo")

import numpy as np

import concourse.bass as bass
import concourse.mybir as mybir
import concourse.tile as tile
from concourse import bacc
from concourse.bass_utils import run_bass_kernel_spmd
from concourse.masks import make_identity

FP32 = mybir.dt.float32
FP32R = mybir.dt.float32r
BF16 = mybir.dt.bfloat16
FP8 = mybir.dt.float8e4
DR = mybir.MatmulPerfMode.DoubleRow

N_TOK = 4096
C = 768
H = 8
HD = 96
C3 = 3 * C
TOK_TILE = 512
N_TILES = N_TOK // TOK_TILE
CHUNKS = TOK_TILE // 128
KK = C // 128
EPS = 1e-12

S_W = 64.0           # w_q/w_k -> fp8 scale
C_QK = 26.0 / 35.5   # qk psum (scaled x64) -> fp8 scale

_CACHED_NC = None


def _vt_strips():
    """Strips (m, p0, run, h, d0): dense partition p of block m holds
    v-feature c = 128m + p = 96h + d -> per-head partition d, head h."""
    strips = []
    for m in range(KK):
        c0 = 128 * m
        p = 0
        while p < 128:
            h, d = divmod(c0 + p, HD)
            run = min(128 - p, HD - d)
            strips.append((m, p, run, h, d))
            p += run
    return strips


def build_nc():
    nc = bacc.Bacc("TRN2", target_bir_lowering=False, debug=False, num_devices=8)

    ident_d = nc.inline_tensor(np.eye(128, dtype=np.float32),
                               name="ident_const").ap()
    x_d = nc.dram_tensor("x", (N_TOK, C), FP32R, kind="ExternalInput").ap()
    wqkv_d = nc.dram_tensor("w_qkv", (C3, C), FP32R, kind="ExternalInput").ap()
    temp_d = nc.dram_tensor("temperature", (H, 1, 1), FP32, kind="ExternalInput").ap()
    wproj_d = nc.dram_tensor("w_proj", (C, C), FP32R, kind="ExternalInput").ap()
    bproj_d = nc.dram_tensor("b_proj", (C,), FP32, kind="ExternalInput").ap()
    out_d = nc.dram_tensor("out", (N_TOK, C), FP32, kind="ExternalOutput").ap()

    with tile.TileContext(nc) as tc:
        _build(tc, nc, x_d, wqkv_d, temp_d, wproj_d, bproj_d, out_d, ident_d)
    nc.compile()
    return nc


def _build(tc, nc, x_d, wqkv_d, temp_d, wproj_d, bproj_d, out_d, ident_d):
    import contextlib

    ctx = contextlib.ExitStack()
    with ctx:
        singles = ctx.enter_context(tc.tile_pool(name="singles", bufs=1))
        dram = ctx.enter_context(tc.tile_pool(name="dram", bufs=1, space="DRAM"))

        # identity loaded from a NEFF-embedded constant on the idle SP
        # HWDGE queue: no Pool time, ready by the time x(0) lands
        ident_f32 = singles.tile([128, 128], FP32)
        nc.sync.dma_start(ident_f32, ident_d)
        ident = singles.tile([128, 128], BF16)
        nc.vector.tensor_copy(ident, ident_f32)
        ident_r = singles.tile([128, 128], FP32R)
        nc.vector.tensor_copy(ident_r, ident_f32)
        ident96f = ident_f32[0:HD, 0:HD]

        # act-table warmup: end on the Sqrt set (phase-2 order is sqrt->exp;
        # a dummy exp there prefetches the Exp set off the critical path)
        warm = singles.tile([1, 1], FP32)
        nc.vector.memset(warm, 0.5)
        nc.scalar.activation(warm, warm, mybir.ActivationFunctionType.Exp)
        nc.scalar.sqrt(warm, warm)

        cg_accum = singles.tile([HD, H, 288], FP32)
        nc.vector.memset(cg_accum, 0.0)
        attn_bf = singles.tile([HD, H, HD], BF16)
        # xT (feature-major bf16 x) for the whole batch element: consumed by
        # the fused projection GEMM at the end, so it persists all of phase 1.
        xT_bf = singles.tile([128, KK, N_TOK], BF16)
        wv_bf = singles.tile([128, KK, C], BF16)      # w_v raw (vfeat-part)
        w_projT = singles.tile([HD, H, C], BF16)      # per-head wp^T

        # ---------------- phase 0 + 1 ----------------
        # w_qk plain layout: w_qk_f8[p][:, kk, j] = 64 * w_qkv[512p+j, 128kk+:]
        qkp_holder = []
        with tc.tile_pool(name="wqk_pool", bufs=1) as wqk_pool, \
             tc.tile_pool(name="xin", bufs=3) as xin, \
             tc.tile_pool(name="xf8", bufs=3) as xf8, \
             tc.tile_pool(name="ps_tr", bufs=2, space="PSUM") as ps_tr, \
             tc.tile_pool(name="ps_one", bufs=6, space="PSUM") as ps_one:
            w_qk_f8 = [wqk_pool.tile([128, KK, 512], FP8, name=f"wqk{p}")
                       for p in range(3)]

            # w_qk loads: fp32r on the two HWDGE queues, parallel with the
            # SWDGE x stream; staging pool scoped to the fill so its SBUF is
            # recycled into qk_t buffers for all 8 tiles
            wload_cm = tc.tile_pool(name="wload", bufs=2)
            wload = wload_cm.__enter__()
            w_blks = {}
            for g in range(6):
                w_blk = wload.tile([128, 2, C], FP32R, name="w_blk",
                                   tag=f"w{g % 2}", bufs=3)
                eng = nc.sync if g % 2 == 0 else nc.scalar
                eng.dma_start(
                    w_blk,
                    wqkv_d[g * 256:(g + 1) * 256, :].rearrange(
                        "(b p) f -> p b f", p=128))
                w_blks[g] = w_blk

            b_all = singles.tile([128, C], FP32)
            nc.sync.dma_start(
                b_all, bass.AP(tensor=bproj_d.tensor, offset=bproj_d.offset,
                               ap=[[0, 128], [1, C]]))
            temp_all = singles.tile([HD, H], FP32)
            nc.sync.dma_start(
                temp_all, bass.AP(tensor=temp_d.tensor, offset=temp_d.offset,
                                  ap=[[0, HD], [1, H]]))

            x_ts = {}

            def issue_x(t):
                """x cast-DMA issue only, so the SWDGE queue never blocks
                behind Pool compute ops."""
                t0 = t * TOK_TILE
                x_t = xin.tile([128, CHUNKS, C], BF16, name="x_t")
                if t < 2:
                    # split by kk-pair so each transpose group starts as its
                    # columns arrive
                    for kp in range(3):
                        nc.gpsimd.dma_start(
                            x_t[:, :, kp * 256:(kp + 1) * 256],
                            x_d[t0:t0 + TOK_TILE,
                                kp * 256:(kp + 1) * 256].rearrange(
                                "(c p) f -> p c f", p=128))
                else:
                    nc.gpsimd.dma_start(
                        x_t, x_d[t0:t0 + TOK_TILE, :].rearrange(
                            "(c p) f -> p c f", p=128))
                x_ts[t] = x_t

            def xstage(t):
                """PE transpose + bf16/fp8 drains for an already-loading x."""
                t0 = t * TOK_TILE
                x_t = x_ts.pop(t)
                xT_f8 = xf8.tile([128, KK, TOK_TILE], FP8, name="xT_f8")
                for kp in range(KK // 2):
                    xps = ps_tr.tile([128, 1024], BF16, name="xps", tag="tr")
                    for j in range(2):
                        kk = 2 * kp + j
                        for c in range(CHUNKS):
                            nc.tensor.transpose(
                                xps[:, j * 512 + c * 128:j * 512 + (c + 1) * 128],
                                x_t[:, c, kk * 128:(kk + 1) * 128], ident)
                    xv = xps.rearrange("p (k f) -> p k f", k=2)
                    nc.vector.tensor_copy(
                        xT_bf[:, 2 * kp:2 * kp + 2, t0:t0 + TOK_TILE], xv)
                    if t < 3:
                        # fill: kp1 on DVE so the first qk chains aren't
                        # ACT-serialized
                        if kp == 1:
                            nc.vector.tensor_copy(
                                xT_f8[:, 2:4, :], xv)
                        else:
                            nc.scalar.copy(
                                xT_f8[:, 2 * kp:2 * kp + 2, :], xv)
                    else:
                        nc.gpsimd.tensor_copy(
                            xT_f8[:, 2 * kp:2 * kp + 2, :],
                            xT_bf[:, 2 * kp:2 * kp + 2, t0:t0 + TOK_TILE])
                return xT_f8

            state = {"qk_pair": []}
            sq = singles.tile([HD, H, 2], FP32)
            scrd = singles.tile([HD, HD], FP32)

            def mmstage(t, xT_f8, mid_hook=None):
                """qk matmuls + drains, covariance every fourth tile.
                12 single-bank psum chains per tile through a 6-deep ring so
                the PE never stalls on a drain."""
                qk_t = qkp_holder[0].tile([128, CHUNKS, 1536], FP8, name="qk_t")

                # comparative advantage: ACT is cheaper on fp8-out psum
                # drains, DVE on bf16 (2x mode) + the cov adds. During fill
                # (t<3) ACT also does the fp8 casts, so DVE takes more.
                if t < 3:
                    dve_drains = {(c, p) for c in range(CHUNKS)
                                  for p in range(3) if (c * 3 + p) % 12 in
                                  (0, 2, 4, 6, 8, 10, 11)}
                else:
                    dve_drains = {(0, 0), (2, 0), (1, 1), (3, 2)}
                for p in range(3):
                    if p == 2 and mid_hook is not None:
                        mid_hook()
                    for c in range(CHUNKS):
                        ps1 = ps_one.tile([128, 512], FP32, name="ps1",
                                          tag="one")
                        for i in range(KK // 2):
                            nc.tensor.matmul(
                                ps1,
                                xT_f8[:, 2 * i:2 * i + 2,
                                      c * 128:(c + 1) * 128],
                                w_qk_f8[p][:, 2 * i:2 * i + 2, :],
                                start=(i == 0), stop=(i == KK // 2 - 1),
                                perf_mode=DR)
                        dst = qk_t[:, c, p * 512:(p + 1) * 512]
                        if (c, p) in dve_drains:
                            nc.vector.tensor_scalar_mul(dst, ps1, C_QK)
                        else:
                            nc.scalar.mul(dst, ps1, C_QK)

                # covariance + Gram over four tiles at a time: per head
                # [Gq | Gk | C] = [q'q | k'k | q'k], DoubleRow chunk pairs
                state["qk_pair"].append(qk_t)
                if t == N_TILES - 1:
                    qk_pair = state["qk_pair"]
                    for h in range(H):
                        cg_ps = ps_one.tile([HD, 288], FP32, name="cg_ps",
                                            tag="one")
                        np_ = 2 * len(qk_pair)
                        for i in range(np_):
                            qkx = qk_pair[i // 2]
                            lo = (i % 2) * 2
                            q_sl = qkx[:, lo:lo + 2, HD * h:HD * h + HD]
                            k_sl = qkx[:, lo:lo + 2,
                                       C + HD * h:C + HD * h + HD]
                            nc.tensor.matmul(
                                cg_ps[:, 0:HD], q_sl, q_sl,
                                start=(i == 0), stop=False, perf_mode=DR)
                            nc.tensor.matmul(
                                cg_ps[:, HD:2 * HD], k_sl, k_sl,
                                start=False, stop=False, perf_mode=DR)
                            nc.tensor.matmul(
                                cg_ps[:, 2 * HD:3 * HD], q_sl, k_sl,
                                start=False, stop=(i == np_ - 1),
                                perf_mode=DR)
                        nc.vector.tensor_add(
                            cg_accum[:, h, :], cg_ps, cg_accum[:, h, :])
                    state["qk_pair"] = []

            def wprep(grp):
                """w_q/w_k rows 256*grp..256*grp+256 -> w_qk_f8 (x64).
                2 kk-blocks (4 transposes) per psum, one merged drain each."""
                w_blk = w_blks.pop(grp)
                goff = (grp % 2) * 256
                for kp in range(KK // 2):
                    tps = ps_one.tile([128, 512], FP32R, name="wps",
                                      tag="one")
                    for j in range(2):
                        kk = 2 * kp + j
                        for b in range(2):
                            nc.tensor.transpose(
                                tps[:, j * 256 + b * 128:
                                    j * 256 + (b + 1) * 128],
                                w_blk[:, b, kk * 128:(kk + 1) * 128], ident_r)
                    dst = w_qk_f8[grp // 2][
                        :, 2 * kp:2 * kp + 2, goff:goff + 256]
                    src = tps.rearrange("p (k f) -> p k f", k=2)
                    if (grp + kp) % 2 == 0:
                        nc.vector.tensor_scalar_mul(dst, src, S_W)
                    else:
                        nc.scalar.mul(dst, src, S_W)

            wp_all = singles.tile([128, KK, C], BF16)

            def wproj_tr(n):
                """w_proj rows 128n..128n+128 -> w_projT[:, :, n*128:...]."""
                wp_blk = wp_all[:, n, :]
                for hg in range(2):
                    ps_w = ps_tr.tile([128, 512], BF16, name="psw", tag="tr")
                    for hh in range(4):
                        h = 4 * hg + hh
                        nc.tensor.transpose(
                            ps_w[0:HD, hh * 128:(hh + 1) * 128],
                            wp_blk[:, h * HD:(h + 1) * HD], ident)
                    src = ps_w[0:HD, :].rearrange("p (h f) -> p h f", h=4)
                    dst = w_projT[:, 4 * hg:4 * hg + 4, n * 128:(n + 1) * 128]
                    if hg == 0:
                        nc.vector.tensor_copy(dst, src)
                    else:
                        nc.scalar.copy(dst, src)

            # x0..x2 issued up front (before any other Pool work) so the
            # first transfer starts as early as possible
            issue_x(0)
            issue_x(1)
            issue_x(2)
            xts = {}
            wprep(0)
            wprep(1)
            xts[0] = xstage(0)
            xts[1] = xstage(1)
            wprep(2)
            wprep(3)
            wprep(4)
            wprep(5)
            wload_cm.__exit__(None, None, None)

            # w staging freed: its SBUF becomes qk_t buffers for all 8
            # tiles, so the covariance runs as one single-round accumulate
            qkp_cm = tc.tile_pool(name="qkp", bufs=8)
            qkp_holder.append(qkp_cm.__enter__())
            for t in range(N_TILES):
                mmstage(t, xts.pop(t))
                if t + 3 < N_TILES:
                    issue_x(t + 3)
                if t == 4:
                    # w_proj load: consumed by the transposes after the loop
                    nc.gpsimd.dma_start(
                        wp_all, wproj_d.rearrange("(b p) f -> p b f", p=128))
                if t == 5:
                    # w_v load: only needed for the M build at the very end
                    nc.gpsimd.dma_start(
                        wv_bf, wqkv_d[2 * C:3 * C, :].rearrange(
                            "(b p) f -> p b f", p=128))
                if t + 2 < N_TILES:
                    xts[t + 2] = xstage(t + 2)

            # w_proj transposes fill the PE gap while the softmax chain runs
            for n in range(KK):
                wproj_tr(n)

            # ---------------- phase 2: norms + per-head softmax ----------
            # k-side norms first: the rk transpose/broadcast chain has the
            # longest latency, so it launches before the q-side is computed
            identb = ident96f[:, None, :].to_broadcast((HD, H, HD))
            scr2 = singles.tile([HD, H, HD], FP32)
            nrm = singles.tile([HD, H, 2], FP32)
            rnorm = singles.tile([HD, H, 2], FP32)
            nc.vector.tensor_tensor(
                scr2, cg_accum[:, :, HD:2 * HD],
                identb, mybir.AluOpType.mult)
            nc.vector.reduce_sum(sq[:, :, 1, None], scr2,
                                 axis=mybir.AxisListType.X)
            nc.scalar.sqrt(nrm[:, :, 1], sq[:, :, 1])
            nc.vector.tensor_scalar_max(nrm[:, :, 1], nrm[:, :, 1], EPS)
            nc.vector.reciprocal(rnorm[:, :, 1], nrm[:, :, 1])

            # each head's rnorm column -> a partition-0 row segment (the
            # broadcast ISA op only reads partition 0)
            rk_ps = [ps_tr.tile([128, 512], FP32, name=f"rk_ps{i}", tag="tr")
                     for i in range(2)]
            for h in range(H):
                nc.tensor.transpose(
                    rk_ps[h // 4][0:1, (h % 4) * HD:(h % 4 + 1) * HD],
                    rnorm[:, h:h + 1, 1], ident96f)

            # q-side norms on DVE while the transpose semaphores propagate
            nc.vector.tensor_tensor(
                scr2, cg_accum[:, :, 0:HD],
                identb, mybir.AluOpType.mult)
            nc.vector.reduce_sum(sq[:, :, 0, None], scr2,
                                 axis=mybir.AxisListType.X)
            nc.scalar.sqrt(nrm[:, :, 0], sq[:, :, 0])
            # prefetch the Exp act table before the per-head exps
            nc.scalar.activation(warm, warm, mybir.ActivationFunctionType.Exp)

            rk_row = singles.tile([1, H * HD], FP32)
            nc.vector.tensor_copy(rk_row[:, 0:4 * HD], rk_ps[0][0:1, 0:4 * HD])
            nc.vector.tensor_copy(rk_row[:, 4 * HD:], rk_ps[1][0:1, 0:4 * HD])
            nc.vector.tensor_scalar_max(nrm[:, :, 0], nrm[:, :, 0], EPS)
            nc.vector.reciprocal(rnorm[:, :, 0], nrm[:, :, 0])
            rq = singles.tile([HD, H], FP32)
            nc.vector.tensor_tensor(rq, rnorm[:, :, 0], temp_all,
                                    mybir.AluOpType.mult)
            rk_all = singles.tile([HD, H, HD], FP32)

            # fully per-head tail: broadcast -> logits -> exp -> normalize,
            # so head 0's B matmuls start while later heads are in flight
            attL = singles.tile([HD, H, HD], FP32)
            sea = singles.tile([HD, H, 1], FP32)
            rsea = singles.tile([HD, H, 1], FP32)
            for h in range(H):
                if h % 4 == 0:
                    nc.gpsimd.partition_broadcast(
                        rk_all[:, h:h + 4, :],
                        rk_row[:, h * HD:(h + 4) * HD])
                nc.vector.tensor_tensor(
                    attL[:, h, :], cg_accum[:, h, 2 * HD:3 * HD],
                    rq[:, h, None].to_broadcast((HD, HD)),
                    mybir.AluOpType.mult)
                nc.vector.tensor_tensor(
                    attL[:, h, :], attL[:, h, :], rk_all[:, h, :],
                    mybir.AluOpType.mult)
                nc.scalar.activation(
                    attL[:, h, :], attL[:, h, :],
                    mybir.ActivationFunctionType.Exp,
                    accum_out=sea[:, h, :])
                nc.vector.reciprocal(rsea[:, h, :], sea[:, h, :])
                nc.vector.tensor_tensor(
                    attn_bf[:, h, :], attL[:, h, :],
                    rsea[:, h, :].to_broadcast((HD, HD)),
                    mybir.AluOpType.mult)
            qkp_cm.__exit__(None, None, None)

        # ---------------- phase 3: B, M = wv^T B, y = x @ M ---------------
        strips = _vt_strips()
        with tc.tile_pool(name="wpp", bufs=1) as wpp, \
             tc.tile_pool(name="yp", bufs=2) as yp, \
             tc.tile_pool(name="ps_b", bufs=2, space="PSUM") as ps_b, \
             tc.tile_pool(name="ps_m", bufs=2, space="PSUM") as ps_m, \
             tc.tile_pool(name="ps_y", bufs=4, space="PSUM") as ps_y:
            # B built directly in dense 128-row layout: per (m, half) one psum
            # whose partition ranges are filled by per-head strip matmuls
            b128 = wpp.tile([128, KK, C], BF16)
            m_sb = wpp.tile([128, KK, C], BF16)
            last_h_of_m = {}
            for (m, p0, run, h, d0) in strips:
                last_h_of_m[m] = h
            def legal_pieces(p0, run, d0):
                """PE col tile positions are restricted: <=32-row tiles can
                sit at 0/32/64/96, <=64 at 0/64, bigger only at 0."""
                if run > 64 and p0 not in (0,):
                    assert p0 == 32 and run == 96
                    return [(32, 32, d0), (64, 64, d0 + 32)]
                if 32 < run <= 64 and p0 not in (0, 64):
                    return [(p0, 32, d0), (p0 + 32, run - 32, d0 + 32)]
                return [(p0, run, d0)]

            for oi, (off, width) in enumerate(((0, 512), (512, 256))):
                open_ps = {}
                for si, (m, p0, run, h, d0) in enumerate(strips):
                    if m not in open_ps:
                        open_ps[m] = ps_b.tile([128, 512], FP32, name="bps")
                    for (pp, rr, dd) in legal_pieces(p0, run, d0):
                        nc.tensor.matmul(
                            open_ps[m][pp:pp + rr, :width],
                            attn_bf[:, h, dd:dd + rr],
                            w_projT[:, h, off:off + width],
                            start=True, stop=True,
                            tile_position=(0, pp))
                    if h == last_h_of_m[m]:
                        bps = open_ps.pop(m)
                        if (m + oi) % 2 == 0:
                            nc.vector.tensor_copy(
                                b128[:, m, off:off + width], bps[:, :width])
                        else:
                            nc.scalar.copy(
                                b128[:, m, off:off + width], bps[:, :width])

                # M half: M = wv^T @ b128 over 6 vfeat blocks
                for n in range(KK):
                    mps = ps_m.tile([128, 512], FP32, name="mps")
                    for b in range(KK):
                        nc.tensor.matmul(
                            mps[:, :width],
                            wv_bf[:, b, n * 128:(n + 1) * 128],
                            b128[:, b, off:off + width],
                            start=(b == 0), stop=(b == KK - 1))
                    if (n + oi) % 2 == 0:
                        nc.vector.tensor_copy(
                            m_sb[:, n, off:off + width], mps[:, :width])
                    else:
                        nc.scalar.copy(
                            m_sb[:, n, off:off + width], mps[:, :width])

            # ---------------- y = x @ M + b ------------------------------
            for t in range(N_TILES):
                for piece in range(2):
                    last = (t == N_TILES - 1 and piece == 1)
                    t0 = t * TOK_TILE + piece * 256
                    y_t = yp.tile([128, 2, C], FP32, name="y_t")
                    for c in range(2):
                        cc = t * CHUNKS + piece * 2 + c
                        for oi, (off, width) in enumerate(
                                ((0, 512), (512, 256))):
                            yps = ps_y.tile([128, 512], FP32, name="yps")
                            for kk in range(KK):
                                nc.tensor.matmul(
                                    yps[:, :width],
                                    xT_bf[:, kk, cc * 128:(cc + 1) * 128],
                                    m_sb[:, kk, off:off + width],
                                    start=(kk == 0), stop=(kk == KK - 1))
                            nc.vector.tensor_tensor(
                                y_t[:, c, off:off + width],
                                yps[:, :width],
                                b_all[:, off:off + width],
                                mybir.AluOpType.add)
                    if last:
                        # split the final store across both HWDGE queues,
                        # one 128-token row-block each, issued per sub-block
                        # so the tail is one drain + one small DMA
                        nc.sync.dma_start(
                            out_d[t0:t0 + 128, :].rearrange(
                                "(c p) f -> p c f", p=128),
                            y_t[:, 0:1, :])
                        nc.scalar.dma_start(
                            out_d[t0 + 128:t0 + 256, :].rearrange(
                                "(c p) f -> p c f", p=128),
                            y_t[:, 1:2, :])
                    else:
                        nc.sync.dma_start(
                            out_d[t0:t0 + 256, :].rearrange(
                                "(c p) f -> p c f", p=128),
                            y_t)


def _get_nc():
    global _CACHED_NC
    if _CACHED_NC is None:
        _CACHED_NC = build_nc()
    return _CACHED_NC


def kernel(x, w_qkv, temperature, w_proj, b_proj):
    nc = _get_nc()
    x = np.ascontiguousarray(np.asarray(x, dtype=np.float32))
    in_maps = []
    for b in range(8):
        in_maps.append({
            "x": x[b],
            "w_qkv": np.asarray(w_qkv, dtype=np.float32),
            "temperature": np.asarray(temperature, dtype=np.float32),
            "w_proj": np.asarray(w_proj, dtype=np.float32),
            "b_proj": np.asarray(b_proj, dtype=np.float32),
        })
    res = run_bass_kernel_spmd(nc, in_maps, core_ids=list(range(8)))
    return np.stack([r["out"] for r in res.results], axis=0)


# revision 77
# speedup vs baseline: 1.0029x; 1.0004x over previous
"""CrossCovarianceAttn Trainium2 kernel.

Data-parallel over B=8 across 8 NeuronCores; each core runs the full model on
one batch element.

Core restructure vs the straightforward pipeline: attn@v and the output
projection are folded all the way back into x.  With
B[(h,e),co] = sum_d attn_h[d,e] * wp[co, 96h+d]  (the per-head attn/proj fold)
and v = Wv x, the output is  y = x @ M,  M = Wv^T @ B  (768x768 per batch).
This deletes the entire v-projection GEMM (768x768x4096) and its psum drains;
the big bf16 GEMM left is y = x @ M (plus the fp8 qk projection).  Wv is
consumed in its raw HBM layout (vfeat on partitions) - no transpose.  B is
built directly in dense 128-row layout by offsetting matmul outputs to the
strip partition ranges, so no DMA relayout is needed.

Numerics: q/k projection and the per-head covariance/Gram matmuls run in
fp8e4m3 with perf_mode=DoubleRow (0.5 cyc/row) - safe because q,k are
l2-normalized over the token dim downstream, which cancels the fp8 scaling,
and softmax logits are small (|logit| <= temperature by Cauchy-Schwarz, so no
max-subtraction).  w_q/w_k are scaled x64 into fp8 range; the qk psum->fp8
cast divides it back via C_QK.  Everything on the v/proj path stays bf16.

Engine layout: PE does transposes + qk (fp8 DR) + covariance (fp8 DR,
accumulated over 4-tile rounds) + B/M builds + the y GEMM.  The 12 qk psum
chains per tile are single-bank through a 6-deep ring so the PE never stalls
on a drain; drains are split DVE/ACT by comparative advantage (DVE gets the
2x-mode bf16 xT drains + cov adds, ACT the fp8 ones), and the steady-state
fp8 xT casts run on the otherwise idle GpSimd engine.  The y bias-add runs
as tensor_tensor-add on DVE (ACT has no tensor_tensor).  x/wproj/wv loads
are GpSimd SWDGE cast-DMAs (fp32->bf16 in flight) with x issued decoupled
from compute so the queue never blocks; w_qk loads are fp32r on the SP/ACT
HWDGE queues (parallel with the x stream) and transposed at 1.5 cyc/row; the
identity comes from a NEFF-embedded constant; stores are issued on SP with
the final piece split across both HWDGE queues.  The rk norm reciprocals
reach the softmax free dim via per-head PE column-transposes to a partition-0
row plus GpSimd partition_broadcast (no DRAM round-trip), and the softmax
tail runs per-head (exp with accum_out) so the B matmuls start while later
heads are still in flight.  ACT's activation-table warmup ends on the Sqrt
set and a dummy Exp prefetches the Exp set off the critical path.
"""
import os
import sys

sys.path.insert(0, "/opt/trn_rl_repo")

import numpy as np

import concourse.bass as bass
import concourse.mybir as mybir
import concourse.tile as tile
from concourse import bacc
from concourse.bass_utils import run_bass_kernel_spmd
from concourse.masks import make_identity

FP32 = mybir.dt.float32
FP32R = mybir.dt.float32r
BF16 = mybir.dt.bfloat16
FP8 = mybir.dt.float8e4
DR = mybir.MatmulPerfMode.DoubleRow

N_TOK = 4096
C = 768
H = 8
HD = 96
C3 = 3 * C
TOK_TILE = 512
N_TILES = N_TOK // TOK_TILE
CHUNKS = TOK_TILE // 128
KK = C // 128
EPS = 1e-12

S_W = 64.0           # w_q/w_k -> fp8 scale
C_QK = 26.0 / 35.5   # qk psum (scaled x64) -> fp8 scale

_CACHED_NC = None


def _vt_strips():
    """Strips (m, p0, run, h, d0): dense partition p of block m holds
    v-feature c = 128m + p = 96h + d -> per-head partition d, head h."""
    strips = []
    for m in range(KK):
        c0 = 128 * m
        p = 0
        while p < 128:
            h, d = divmod(c0 + p, HD)
            run = min(128 - p, HD - d)
            strips.append((m, p, run, h, d))
            p += run
    return strips


def build_nc():
    nc = bacc.Bacc("TRN2", target_bir_lowering=False, debug=False, num_devices=8)

    ident_d = nc.inline_tensor(np.eye(128, dtype=np.float32),
                               name="ident_const").ap()
    x_d = nc.dram_tensor("x", (N_TOK, C), FP32R, kind="ExternalInput").ap()
    wqkv_d = nc.dram_tensor("w_qkv", (C3, C), FP32R, kind="ExternalInput").ap()
    temp_d = nc.dram_tensor("temperature", (H, 1, 1), FP32, kind="ExternalInput").ap()
    wproj_d = nc.dram_tensor("w_proj", (C, C), FP32R, kind="ExternalInput").ap()
    bproj_d = nc.dram_tensor("b_proj", (C,), FP32, kind="ExternalInput").ap()
    out_d = nc.dram_tensor("out", (N_TOK, C), FP32, kind="ExternalOutput").ap()

    with tile.TileContext(nc) as tc:
        _build(tc, nc, x_d, wqkv_d, temp_d, wproj_d, bproj_d, out_d, ident_d)
    nc.compile()
    return nc


def _build(tc, nc, x_d, wqkv_d, temp_d, wproj_d, bproj_d, out_d, ident_d):
    import contextlib

    ctx = contextlib.ExitStack()
    with ctx:
        singles = ctx.enter_context(tc.tile_pool(name="singles", bufs=1))
        dram = ctx.enter_context(tc.tile_pool(name="dram", bufs=1, space="DRAM"))

        # identity loaded from a NEFF-embedded constant on the idle SP
        # HWDGE queue: no Pool time, ready by the time x(0) lands
        ident_f32 = singles.tile([128, 128], FP32)
        nc.sync.dma_start(ident_f32, ident_d)
        ident = singles.tile([128, 128], BF16)
        nc.vector.tensor_copy(ident, ident_f32)
        ident_r = singles.tile([128, 128], FP32R)
        nc.vector.tensor_copy(ident_r, ident_f32)
        ident96f = ident_f32[0:HD, 0:HD]

        # act-table warmup: end on the Sqrt set (phase-2 order is sqrt->exp;
        # a dummy exp there prefetches the Exp set off the critical path)
        warm = singles.tile([1, 1], FP32)
        nc.vector.memset(warm, 0.5)
        nc.scalar.activation(warm, warm, mybir.ActivationFunctionType.Exp)
        nc.scalar.sqrt(warm, warm)

        cg_accum = singles.tile([HD, H, 288], FP32)
        attn_bf = singles.tile([HD, H, HD], BF16)
        # xT (feature-major bf16 x) for the whole batch element: consumed by
        # the fused projection GEMM at the end, so it persists all of phase 1.
        xT_bf = singles.tile([128, KK, N_TOK], BF16)
        wv_bf = singles.tile([128, KK, C], BF16)      # w_v raw (vfeat-part)
        w_projT = singles.tile([HD, H, C], BF16)      # per-head wp^T

        # ---------------- phase 0 + 1 ----------------
        # w_qk plain layout: w_qk_f8[p][:, kk, j] = 64 * w_qkv[512p+j, 128kk+:]
        qkp_holder = []
        with tc.tile_pool(name="wqk_pool", bufs=1) as wqk_pool, \
             tc.tile_pool(name="xin", bufs=3) as xin, \
             tc.tile_pool(name="xf8", bufs=3) as xf8, \
             tc.tile_pool(name="ps_tr", bufs=2, space="PSUM") as ps_tr, \
             tc.tile_pool(name="ps_one", bufs=6, space="PSUM") as ps_one:
            w_qk_f8 = [wqk_pool.tile([128, KK, 512], FP8, name=f"wqk{p}")
                       for p in range(3)]

            # w_qk loads: fp32r on the two HWDGE queues, parallel with the
            # SWDGE x stream; staging pool scoped to the fill so its SBUF is
            # recycled into qk_t buffers for all 8 tiles
            wload_cm = tc.tile_pool(name="wload", bufs=2)
            wload = wload_cm.__enter__()
            w_blks = {}
            for g in range(6):
                w_blk = wload.tile([128, 2, C], FP32R, name="w_blk",
                                   tag=f"w{g % 2}", bufs=3)
                eng = nc.sync if g % 2 == 0 else nc.scalar
                eng.dma_start(
                    w_blk,
                    wqkv_d[g * 256:(g + 1) * 256, :].rearrange(
                        "(b p) f -> p b f", p=128))
                w_blks[g] = w_blk

            b_all = singles.tile([128, C], FP32)
            nc.sync.dma_start(
                b_all, bass.AP(tensor=bproj_d.tensor, offset=bproj_d.offset,
                               ap=[[0, 128], [1, C]]))
            temp_all = singles.tile([HD, H], FP32)
            nc.sync.dma_start(
                temp_all, bass.AP(tensor=temp_d.tensor, offset=temp_d.offset,
                                  ap=[[0, HD], [1, H]]))

            x_ts = {}

            def issue_x(t):
                """x cast-DMA issue only, so the SWDGE queue never blocks
                behind Pool compute ops."""
                t0 = t * TOK_TILE
                x_t = xin.tile([128, CHUNKS, C], BF16, name="x_t")
                if t < 2:
                    # split by kk-pair so each transpose group starts as its
                    # columns arrive
                    for kp in range(3):
                        nc.gpsimd.dma_start(
                            x_t[:, :, kp * 256:(kp + 1) * 256],
                            x_d[t0:t0 + TOK_TILE,
                                kp * 256:(kp + 1) * 256].rearrange(
                                "(c p) f -> p c f", p=128))
                else:
                    nc.gpsimd.dma_start(
                        x_t, x_d[t0:t0 + TOK_TILE, :].rearrange(
                            "(c p) f -> p c f", p=128))
                x_ts[t] = x_t

            def xstage(t):
                """PE transpose + bf16/fp8 drains for an already-loading x."""
                t0 = t * TOK_TILE
                x_t = x_ts.pop(t)
                xT_f8 = xf8.tile([128, KK, TOK_TILE], FP8, name="xT_f8")
                for kp in range(KK // 2):
                    xps = ps_tr.tile([128, 1024], BF16, name="xps", tag="tr")
                    for j in range(2):
                        kk = 2 * kp + j
                        for c in range(CHUNKS):
                            nc.tensor.transpose(
                                xps[:, j * 512 + c * 128:j * 512 + (c + 1) * 128],
                                x_t[:, c, kk * 128:(kk + 1) * 128], ident)
                    xv = xps.rearrange("p (k f) -> p k f", k=2)
                    nc.vector.tensor_copy(
                        xT_bf[:, 2 * kp:2 * kp + 2, t0:t0 + TOK_TILE], xv)
                    if t < 3:
                        # fill: kp1 on DVE so the first qk chains aren't
                        # ACT-serialized
                        if kp == 1:
                            nc.vector.tensor_copy(
                                xT_f8[:, 2:4, :], xv)
                        else:
                            nc.scalar.copy(
                                xT_f8[:, 2 * kp:2 * kp + 2, :], xv)
                    else:
                        nc.gpsimd.tensor_copy(
                            xT_f8[:, 2 * kp:2 * kp + 2, :],
                            xT_bf[:, 2 * kp:2 * kp + 2, t0:t0 + TOK_TILE])
                return xT_f8

            state = {"qk_pair": []}
            sq = singles.tile([HD, H, 2], FP32)
            scrd = singles.tile([HD, HD], FP32)

            def mmstage(t, xT_f8, mid_hook=None):
                """qk matmuls + drains, covariance every fourth tile.
                12 single-bank psum chains per tile through a 6-deep ring so
                the PE never stalls on a drain."""
                qk_t = qkp_holder[0].tile([128, CHUNKS, 1536], FP8, name="qk_t")

                # comparative advantage: ACT is cheaper on fp8-out psum
                # drains, DVE on bf16 (2x mode) + the cov adds. During fill
                # (t<3) ACT also does the fp8 casts, so DVE takes more.
                if t < 3:
                    dve_drains = {(c, p) for c in range(CHUNKS)
                                  for p in range(3) if (c * 3 + p) % 12 in
                                  (0, 2, 4, 6, 8, 10, 11)}
                else:
                    dve_drains = {(0, 0), (2, 0), (1, 1), (3, 2)}
                for p in range(3):
                    if p == 2 and mid_hook is not None:
                        mid_hook()
                    for c in range(CHUNKS):
                        ps1 = ps_one.tile([128, 512], FP32, name="ps1",
                                          tag="one")
                        for i in range(KK // 2):
                            nc.tensor.matmul(
                                ps1,
                                xT_f8[:, 2 * i:2 * i + 2,
                                      c * 128:(c + 1) * 128],
                                w_qk_f8[p][:, 2 * i:2 * i + 2, :],
                                start=(i == 0), stop=(i == KK // 2 - 1),
                                perf_mode=DR)
                        dst = qk_t[:, c, p * 512:(p + 1) * 512]
                        if (c, p) in dve_drains:
                            nc.vector.tensor_scalar_mul(dst, ps1, C_QK)
                        else:
                            nc.scalar.mul(dst, ps1, C_QK)

                # covariance + Gram over four tiles at a time: per head
                # [Gq | Gk | C] = [q'q | k'k | q'k], DoubleRow chunk pairs
                state["qk_pair"].append(qk_t)
                if t == N_TILES - 1:
                    qk_pair = state["qk_pair"]
                    for h in range(H):
                        cg_ps = ps_one.tile([HD, 288], FP32, name="cg_ps",
                                            tag="one")
                        np_ = 2 * len(qk_pair)
                        for i in range(np_):
                            qkx = qk_pair[i // 2]
                            lo = (i % 2) * 2
                            q_sl = qkx[:, lo:lo + 2, HD * h:HD * h + HD]
                            k_sl = qkx[:, lo:lo + 2,
                                       C + HD * h:C + HD * h + HD]
                            nc.tensor.matmul(
                                cg_ps[:, 0:HD], q_sl, q_sl,
                                start=(i == 0), stop=False, perf_mode=DR)
                            nc.tensor.matmul(
                                cg_ps[:, HD:2 * HD], k_sl, k_sl,
                                start=False, stop=False, perf_mode=DR)
                            nc.tensor.matmul(
                                cg_ps[:, 2 * HD:3 * HD], q_sl, k_sl,
                                start=False, stop=(i == np_ - 1),
                                perf_mode=DR)
                        if h % 2 == 0:
                            nc.vector.tensor_copy(cg_accum[:, h, :], cg_ps)
                        else:
                            nc.scalar.copy(cg_accum[:, h, :], cg_ps)
                    state["qk_pair"] = []

            def wprep(grp):
                """w_q/w_k rows 256*grp..256*grp+256 -> w_qk_f8 (x64).
                2 kk-blocks (4 transposes) per psum, one merged drain each."""
                w_blk = w_blks.pop(grp)
                goff = (grp % 2) * 256
                for kp in range(KK // 2):
                    tps = ps_one.tile([128, 512], FP32R, name="wps",
                                      tag="one")
                    for j in range(2):
                        kk = 2 * kp + j
                        for b in range(2):
                            nc.tensor.transpose(
                                tps[:, j * 256 + b * 128:
                                    j * 256 + (b + 1) * 128],
                                w_blk[:, b, kk * 128:(kk + 1) * 128], ident_r)
                    dst = w_qk_f8[grp // 2][
                        :, 2 * kp:2 * kp + 2, goff:goff + 256]
                    src = tps.rearrange("p (k f) -> p k f", k=2)
                    if (grp + kp) % 2 == 0:
                        nc.vector.tensor_scalar_mul(dst, src, S_W)
                    else:
                        nc.scalar.mul(dst, src, S_W)

            wp_all = singles.tile([128, KK, C], BF16)

            def wproj_tr(n):
                """w_proj rows 128n..128n+128 -> w_projT[:, :, n*128:...]."""
                wp_blk = wp_all[:, n, :]
                for hg in range(2):
                    ps_w = ps_tr.tile([128, 512], BF16, name="psw", tag="tr")
                    for hh in range(4):
                        h = 4 * hg + hh
                        nc.tensor.transpose(
                            ps_w[0:HD, hh * 128:(hh + 1) * 128],
                            wp_blk[:, h * HD:(h + 1) * HD], ident)
                    src = ps_w[0:HD, :].rearrange("p (h f) -> p h f", h=4)
                    dst = w_projT[:, 4 * hg:4 * hg + 4, n * 128:(n + 1) * 128]
                    if hg == 0:
                        nc.vector.tensor_copy(dst, src)
                    else:
                        nc.scalar.copy(dst, src)

            # x0..x2 issued up front (before any other Pool work) so the
            # first transfer starts as early as possible
            issue_x(0)
            issue_x(1)
            issue_x(2)
            xts = {}
            wprep(0)
            wprep(1)
            xts[0] = xstage(0)
            xts[1] = xstage(1)
            wprep(2)
            wprep(3)
            wprep(4)
            wprep(5)
            wload_cm.__exit__(None, None, None)

            # w staging freed: its SBUF becomes qk_t buffers for all 8
            # tiles, so the covariance runs as one single-round accumulate
            qkp_cm = tc.tile_pool(name="qkp", bufs=8)
            qkp_holder.append(qkp_cm.__enter__())
            for t in range(N_TILES):
                mmstage(t, xts.pop(t))
                if t + 3 < N_TILES:
                    issue_x(t + 3)
                if t == 4:
                    # w_proj load: consumed by the transposes after the loop
                    nc.gpsimd.dma_start(
                        wp_all, wproj_d.rearrange("(b p) f -> p b f", p=128))
                if t == 5:
                    # w_v load: only needed for the M build at the very end
                    nc.gpsimd.dma_start(
                        wv_bf, wqkv_d[2 * C:3 * C, :].rearrange(
                            "(b p) f -> p b f", p=128))
                if t + 2 < N_TILES:
                    xts[t + 2] = xstage(t + 2)

            # w_proj transposes fill the PE gap while the softmax chain runs
            for n in range(KK):
                wproj_tr(n)

            # ---------------- phase 2: norms + per-head softmax ----------
            # k-side norms first: the rk transpose/broadcast chain has the
            # longest latency, so it launches before the q-side is computed
            identb = ident96f[:, None, :].to_broadcast((HD, H, HD))
            scr2 = singles.tile([HD, H, HD], FP32)
            nrm = singles.tile([HD, H, 2], FP32)
            rnorm = singles.tile([HD, H, 2], FP32)
            nc.vector.tensor_tensor(
                scr2, cg_accum[:, :, HD:2 * HD],
                identb, mybir.AluOpType.mult)
            nc.vector.reduce_sum(sq[:, :, 1, None], scr2,
                                 axis=mybir.AxisListType.X)
            nc.scalar.sqrt(nrm[:, :, 1], sq[:, :, 1])
            nc.vector.tensor_scalar_max(nrm[:, :, 1], nrm[:, :, 1], EPS)
            nc.vector.reciprocal(rnorm[:, :, 1], nrm[:, :, 1])

            # each head's rnorm column -> a partition-0 row segment (the
            # broadcast ISA op only reads partition 0)
            rk_ps = [ps_tr.tile([128, 512], FP32, name=f"rk_ps{i}", tag="tr")
                     for i in range(2)]
            for h in range(H):
                nc.tensor.transpose(
                    rk_ps[h // 4][0:1, (h % 4) * HD:(h % 4 + 1) * HD],
                    rnorm[:, h:h + 1, 1], ident96f)

            # q-side norms on DVE while the transpose semaphores propagate
            nc.vector.tensor_tensor(
                scr2, cg_accum[:, :, 0:HD],
                identb, mybir.AluOpType.mult)
            nc.vector.reduce_sum(sq[:, :, 0, None], scr2,
                                 axis=mybir.AxisListType.X)
            nc.scalar.sqrt(nrm[:, :, 0], sq[:, :, 0])
            # prefetch the Exp act table before the per-head exps
            nc.scalar.activation(warm, warm, mybir.ActivationFunctionType.Exp)

            rk_row = singles.tile([1, H * HD], FP32)
            nc.vector.tensor_copy(rk_row[:, 0:4 * HD], rk_ps[0][0:1, 0:4 * HD])
            nc.vector.tensor_copy(rk_row[:, 4 * HD:], rk_ps[1][0:1, 0:4 * HD])
            nc.vector.tensor_scalar_max(nrm[:, :, 0], nrm[:, :, 0], EPS)
            nc.vector.reciprocal(rnorm[:, :, 0], nrm[:, :, 0])
            rq = singles.tile([HD, H], FP32)
            nc.vector.tensor_tensor(rq, rnorm[:, :, 0], temp_all,
                                    mybir.AluOpType.mult)
            rk_all = singles.tile([HD, H, HD], FP32)

            # fully per-head tail: broadcast -> logits -> exp -> normalize,
            # so head 0's B matmuls start while later heads are in flight
            attL = singles.tile([HD, H, HD], FP32)
            sea = singles.tile([HD, H, 1], FP32)
            rsea = singles.tile([HD, H, 1], FP32)
            for h in range(H):
                if h % 4 == 0:
                    nc.gpsimd.partition_broadcast(
                        rk_all[:, h:h + 4, :],
                        rk_row[:, h * HD:(h + 4) * HD])
                nc.vector.tensor_tensor(
                    attL[:, h, :], cg_accum[:, h, 2 * HD:3 * HD],
                    rq[:, h, None].to_broadcast((HD, HD)),
                    mybir.AluOpType.mult)
                nc.vector.tensor_tensor(
                    attL[:, h, :], attL[:, h, :], rk_all[:, h, :],
                    mybir.AluOpType.mult)
                nc.scalar.activation(
                    attL[:, h, :], attL[:, h, :],
                    mybir.ActivationFunctionType.Exp,
                    accum_out=sea[:, h, :])
                nc.vector.reciprocal(rsea[:, h, :], sea[:, h, :])
                nc.vector.tensor_tensor(
                    attn_bf[:, h, :], attL[:, h, :],
                    rsea[:, h, :].to_broadcast((HD, HD)),
                    mybir.AluOpType.mult)
            qkp_cm.__exit__(None, None, None)

        # ---------------- phase 3: B, M = wv^T B, y = x @ M ---------------
        strips = _vt_strips()
        with tc.tile_pool(name="wpp", bufs=1) as wpp, \
             tc.tile_pool(name="yp", bufs=2) as yp, \
             tc.tile_pool(name="ps_b", bufs=2, space="PSUM") as ps_b, \
             tc.tile_pool(name="ps_m", bufs=2, space="PSUM") as ps_m, \
             tc.tile_pool(name="ps_y", bufs=4, space="PSUM") as ps_y:
            # B built directly in dense 128-row layout: per (m, half) one psum
            # whose partition ranges are filled by per-head strip matmuls
            b128 = wpp.tile([128, KK, C], BF16)
            m_sb = wpp.tile([128, KK, C], BF16)
            last_h_of_m = {}
            for (m, p0, run, h, d0) in strips:
                last_h_of_m[m] = h
            def legal_pieces(p0, run, d0):
                """PE col tile positions are restricted: <=32-row tiles can
                sit at 0/32/64/96, <=64 at 0/64, bigger only at 0."""
                if run > 64 and p0 not in (0,):
                    assert p0 == 32 and run == 96
                    return [(32, 32, d0), (64, 64, d0 + 32)]
                if 32 < run <= 64 and p0 not in (0, 64):
                    return [(p0, 32, d0), (p0 + 32, run - 32, d0 + 32)]
                return [(p0, run, d0)]

            for oi, (off, width) in enumerate(((0, 512), (512, 256))):
                open_ps = {}
                for si, (m, p0, run, h, d0) in enumerate(strips):
                    if m not in open_ps:
                        open_ps[m] = ps_b.tile([128, 512], FP32, name="bps")
                    for (pp, rr, dd) in legal_pieces(p0, run, d0):
                        nc.tensor.matmul(
                            open_ps[m][pp:pp + rr, :width],
                            attn_bf[:, h, dd:dd + rr],
                            w_projT[:, h, off:off + width],
                            start=True, stop=True,
                            tile_position=(0, pp))
                    if h == last_h_of_m[m]:
                        bps = open_ps.pop(m)
                        if (m + oi) % 2 == 0:
                            nc.vector.tensor_copy(
                                b128[:, m, off:off + width], bps[:, :width])
                        else:
                            nc.scalar.copy(
                                b128[:, m, off:off + width], bps[:, :width])

                # M half: M = wv^T @ b128 over 6 vfeat blocks
                for n in range(KK):
                    mps = ps_m.tile([128, 512], FP32, name="mps")
                    for b in range(KK):
                        nc.tensor.matmul(
                            mps[:, :width],
                            wv_bf[:, b, n * 128:(n + 1) * 128],
                            b128[:, b, off:off + width],
                            start=(b == 0), stop=(b == KK - 1))
                    if (n + oi) % 2 == 0:
                        nc.vector.tensor_copy(
                            m_sb[:, n, off:off + width], mps[:, :width])
                    else:
                        nc.scalar.copy(
                            m_sb[:, n, off:off + width], mps[:, :width])

            # ---------------- y = x @ M + b ------------------------------
            for t in range(N_TILES):
                for piece in range(2):
                    last = (t == N_TILES - 1 and piece == 1)
                    t0 = t * TOK_TILE + piece * 256
                    y_t = yp.tile([128, 2, C], FP32, name="y_t")
                    for c in range(2):
                        cc = t * CHUNKS + piece * 2 + c
                        for oi, (off, width) in enumerate(
                                ((0, 512), (512, 256))):
                            yps = ps_y.tile([128, 512], FP32, name="yps")
                            for kk in range(KK):
                                nc.tensor.matmul(
                                    yps[:, :width],
                                    xT_bf[:, kk, cc * 128:(cc + 1) * 128],
                                    m_sb[:, kk, off:off + width],
                                    start=(kk == 0), stop=(kk == KK - 1))
                            nc.vector.tensor_tensor(
                                y_t[:, c, off:off + width],
                                yps[:, :width],
                                b_all[:, off:off + width],
                                mybir.AluOpType.add)
                    if last:
                        # split the final store across both HWDGE queues,
                        # one 128-token row-block each, issued per sub-block
                        # so the tail is one drain + one small DMA
                        nc.sync.dma_start(
                            out_d[t0:t0 + 128, :].rearrange(
                                "(c p) f -> p c f", p=128),
                            y_t[:, 0:1, :])
                        nc.scalar.dma_start(
                            out_d[t0 + 128:t0 + 256, :].rearrange(
                                "(c p) f -> p c f", p=128),
                            y_t[:, 1:2, :])
                    else:
                        nc.sync.dma_start(
                            out_d[t0:t0 + 256, :].rearrange(
                                "(c p) f -> p c f", p=128),
                            y_t)


def _get_nc():
    global _CACHED_NC
    if _CACHED_NC is None:
        _CACHED_NC = build_nc()
    return _CACHED_NC


def kernel(x, w_qkv, temperature, w_proj, b_proj):
    nc = _get_nc()
    x = np.ascontiguousarray(np.asarray(x, dtype=np.float32))
    in_maps = []
    for b in range(8):
        in_maps.append({
            "x": x[b],
            "w_qkv": np.asarray(w_qkv, dtype=np.float32),
            "temperature": np.asarray(temperature, dtype=np.float32),
            "w_proj": np.asarray(w_proj, dtype=np.float32),
            "b_proj": np.asarray(b_proj, dtype=np.float32),
        })
    res = run_bass_kernel_spmd(nc, in_maps, core_ids=list(range(8)))
    return np.stack([r["out"] for r in res.results], axis=0)
